# revision 54
# baseline (speedup 1.0000x reference)
"""Trainium2 Bass kernel for ExtractRelevantPatches (pool -> top-k -> gather).

Full-input contract: kernel(heatmap [64,448,448,1] f32, image [64,448,448,3] f32)
-> [1344, 64, 64, 3] f32.

Sharding: pure data-parallel over batch; 8 batches per NeuronCore, 8 cores.

Pipelined per-core algorithm (raw Bass, explicit semaphores), 4 blocks of
2 batches, so gather descriptor expansion + HBM flow of early blocks overlap
pool/top-k of later blocks:

  Per block g (batches 2g, 2g+1):
  1. L_g: batch 2g on the sync HWDGE queue -> SBUF partitions 0-63, batch
     2g+1 on the scalar HWDGE queue -> partitions 64-127 (concurrent
     64-partition streams; SBUF write bw is ~2.9 GB/s/partition).
  2. R_g (DVE): reduce_sum over 64-col groups -> red_g [128, 7, 7].
  3. P_g (TensorE): two single-column matmuls with the 0/1 matrix G64
     (G64[p,b]=1 iff p//64==b) -> psV_g [1, 98]: BOTH batches' 49 pooled
     sums side by side on partition 0.  Ranking by sums == by means.
  4. T_g (DVE): per-batch top-24 on [1,49] slices, the two batches' rounds
     interleaved (max8 / find_index8 / match_replace8 x3); keep first 21
     (descending, matching jax top_k).  idx_f [1, 42] = cast of both idx
     lists, b-major -- already flat on partition 0, so no cross-partition
     flatten DMA is needed anywhere.
  5. A_g (Act): u = (idx + 0.5)*(1/7) - 0.5 (scale+bias in one activation).
  6. C_g (DVE): br = cast_i32(u) (round-to-nearest == floor after the -0.5
     bias); br_f = cast_f32(br).  No DVE integer multiplies anywhere (int
     TENSOR_SCALAR mult stalls ~7.5us while gpsimd descriptor expansion
     runs).
  7. B_g (TensorE): three accumulating broadcast matmuls:
     psD_g[128, 168] = 1*idx_f + 441*br_f (x4 along free via to_broadcast)
     + statc row (the s-dependent static part 112*(s%4) + 3136*b).
  8. cp_g (Act): idx16 slice = cast_i16(psD_g + 7*(p%16)) via bias AP.
     gpsimd cannot read PSUM; the Act engine can.
  9. G_g (gpsimd SWDGE): 4 dma_gather calls (768/768/640/512 idxs) on
     queues 0-3 -> all four SWDGE core-pairs expand descriptors
     concurrently (~9ns/desc per pair; the pipeline's pacing resource).
 10. S_gc: per-call stores GT cols -> DRAM out on the sync HWDGE queue.
"""

import numpy as np

_N_CORES = 8
_B = 64
_B_LOC = _B // _N_CORES  # 8
_PATCH = 64
_GRID = 7
_NPATCH = 21
_PROW = _PATCH * 3            # 192 elements per patch-row
_OUT_ROWS_LOC = _B_LOC * _NPATCH  # 168
_NBLK = 4                     # blocks of 2 batches
_CALL_COLS = [6, 6, 5, 4]     # gather call sizes per block, in 128-row cols
_CALL_OFF = [0, 6, 12, 17]

_nc_cache = None


def build_program():
    """Build the per-core SPMD Bass program (cached)."""
    global _nc_cache
    if _nc_cache is not None:
        return _nc_cache

    import concourse.bass as bass
    import concourse.bacc as bacc
    import concourse.mybir as mybir

    f32 = mybir.dt.float32
    i16 = mybir.dt.int16
    i32 = mybir.dt.int32
    u32 = mybir.dt.uint32
    X = mybir.AxisListType.X
    Op = mybir.AluOpType
    Act = mybir.ActivationFunctionType

    nc = bacc.Bacc(num_swdge_queues=4)

    hm_in = nc.declare_dram_parameter(
        "heatmap", [_B_LOC, 448, 448, 1], f32, isOutput=False)
    img_in = nc.declare_dram_parameter(
        "image", [_B_LOC, 448, 448, 3], f32, isOutput=False)
    out_t = nc.declare_dram_parameter(
        "out", [_OUT_ROWS_LOC, _PATCH, _PATCH, 3], f32, isOutput=True)

    # Static parts of the gather index list: position i = R at [R%16, R//16],
    # R = 16*s + w; static term = 112*(s%4) + 7*w + 3136*(R//1344).
    # The s-dependent part rides the third accumulating matmul (statc row
    # per block); the w-dependent 7*(p%16) part rides the Act copy's bias.
    s_ar = np.arange(84, dtype=np.int64)
    stat = np.zeros((1, 672), dtype=np.float32)
    for m_ in range(8):
        stat[0, 84 * m_:84 * m_ + 84] = 112 * (s_ar % 4) + 3136 * m_
    statc_const = nc.inline_tensor(stat, name="statc_const")
    b7 = (7.0 * (np.arange(128) % 16)).reshape(128, 1).astype(np.float32)
    bias7w_const = nc.inline_tensor(b7, name="bias7w_const")

    # heatmap block view [128, 4, 7, 448]: within block g, row R2 = 448*bp+r
    # sits at partition R2%128, free (R2//128, col) -- one full-rate
    # 128-partition DMA per 2-batch block
    hm_blk = (hm_in[:]
              .rearrange("(g two) r c one -> g (two r) (c one)", two=2)
              .rearrange("g (n p) c -> p g n c", p=128))

    # image patch-row view: [25088, 192]
    img_rows = (img_in[:]
                .rearrange("b r c ch -> (b r c ch)")
                .rearrange("(n e) -> n e", e=_PROW))

    # output patch-row view [10752, 192] -> [p, c, e] with R = 128*c + p
    out_pc = (out_t[:]
              .rearrange("r a b c -> (r a b c)")
              .rearrange("(n e) -> n e", e=_PROW)
              .rearrange("(c p) e -> p c e", p=128))

    from contextlib import ExitStack

    with ExitStack() as ctx:
        e = ctx.enter_context
        hmB = [e(nc.sbuf_tensor(f"hm{g}", [128, 7, 448], f32))
               for g in range(_NBLK)]
        red = [e(nc.sbuf_tensor(f"red{g}", [128, 7, 7], f32))
               for g in range(_NBLK)]
        G64 = e(nc.sbuf_tensor("G64", [128, 2], f32))
        ones128 = e(nc.sbuf_tensor("ones128", [1, 128], f32))
        sel1 = [e(nc.sbuf_tensor(f"sel1_{b}", [33, 128], f32))
                for b in range(2)]
        sel441 = [e(nc.sbuf_tensor(f"sel441_{b}", [33, 128], f32))
                  for b in range(2)]
        Vt = e(nc.sbuf_tensor("Vt", [33, 49], f32))
        vw = [e(nc.sbuf_tensor(f"vw{i}", [33, 49], f32)) for i in range(2)]
        m2 = e(nc.sbuf_tensor("m2", [33, 8], f32))
        idxu = [e(nc.sbuf_tensor(f"idxu{g}", [33, 24], u32))
                for g in range(_NBLK)]
        idx_f = [e(nc.sbuf_tensor(f"idx_f{g}", [33, _NPATCH], f32))
                 for g in range(_NBLK)]
        u_f = [e(nc.sbuf_tensor(f"u_f{g}", [33, _NPATCH], f32))
               for g in range(_NBLK)]
        br_i = [e(nc.sbuf_tensor(f"br_i{g}", [33, _NPATCH], i32))
                for g in range(_NBLK)]
        br_f = [e(nc.sbuf_tensor(f"br_f{g}", [33, _NPATCH], f32))
                for g in range(_NBLK)]
        bias07 = e(nc.sbuf_tensor("bias07", [33, 1], f32))
        statc = e(nc.sbuf_tensor("statc", [1, 672], f32))
        bias7w = e(nc.sbuf_tensor("bias7w", [128, 1], f32))
        idx16 = e(nc.sbuf_tensor("idx16", [128, 672], i16))
        warmidx = e(nc.sbuf_tensor("warmidx", [128, 8], i16))
        GT = e(nc.sbuf_tensor("GT", [128, 84, _PROW], f32))
        GTwarm = e(nc.sbuf_tensor("GTwarm", [128, 1, _PROW], f32))
        psV = [e(nc.psum_tensor(f"psV{g}", [33, 64], f32))
               for g in range(_NBLK)]
        psD = [e(nc.psum_tensor(f"psD{g}", [128, 168], f32))
               for g in range(_NBLK)]
        s_loadE = e(nc.semaphore("s_loadE"))
        s_loadO = e(nc.semaphore("s_loadO"))
        s_stt = e(nc.semaphore("s_stt"))
        s_red = e(nc.semaphore("s_red"))
        s_pmm = e(nc.semaphore("s_pmm"))
        s_topk = e(nc.semaphore("s_topk"))
        s_act = e(nc.semaphore("s_act"))
        s_brf = e(nc.semaphore("s_brf"))
        s_psd = e(nc.semaphore("s_psd"))
        s_bmm = e(nc.semaphore("s_bmm"))
        s_ones = e(nc.semaphore("s_ones"))
        s_warm = e(nc.semaphore("s_warm"))
        s_gc = [e(nc.semaphore(f"s_gc{k}")) for k in range(_NBLK)]
        s_st = e(nc.semaphore("s_st"))
        block = e(nc.Block())

        @block.sync
        def _(sync):
            sync.dma_start(
                out=hmB[0][:, 0:4, :].rearrange(
                    "p (one rhi) c -> p one rhi c", one=1),
                in_=hm_blk[:, 0:1, 0:4, :],
            ).then_inc(s_loadE, 16)
            sync.dma_start(
                out=hmB[0][:, 4:7, :].rearrange(
                    "p (one rhi) c -> p one rhi c", one=1),
                in_=hm_blk[:, 0:1, 4:7, :],
            ).then_inc(s_loadE, 16)
            sync.dma_start(
                out=hmB[2][:].rearrange("p (one rhi) c -> p one rhi c",
                                        one=1),
                in_=hm_blk[:, 2:3, :, :],
            ).then_inc(s_loadE, 16)
            for g in (0, 2):
                sync.wait_ge(s_gc[g], 64)
                sync.dma_start(
                    out=out_pc[:, 21 * g:21 * g + 21, :],
                    in_=GT[:, 21 * g:21 * g + 21, :],
                ).then_inc(s_st, 16)
            sync.wait_ge(s_st, 64)

        @block.scalar
        def _(sc):
            sc.dma_start(
                out=hmB[1][:].rearrange("p (one rhi) c -> p one rhi c",
                                        one=1),
                in_=hm_blk[:, 1:2, :, :],
            ).then_inc(s_loadO, 16)
            sc.dma_start(out=statc[:], in_=statc_const[:]).then_inc(s_stt, 16)
            sc.dma_start(
                out=bias7w[:], in_=bias7w_const[:]).then_inc(s_stt, 16)
            sc.dma_start(
                out=hmB[3][:].rearrange("p (one rhi) c -> p one rhi c",
                                        one=1),
                in_=hm_blk[:, 3:4, :, :],
            ).then_inc(s_loadO, 16)
            for g in range(_NBLK):
                # DVE dtype-converting ops stall behind gpsimd descriptor
                # expansion, so ALL casts live on the Act engine:
                # idx_f = cast_f32(idxu); u = (idx+0.5)/7 - 0.5;
                # br = cast_i32(u) (round == floor after the -0.5 bias);
                # br_f = cast_f32(br)
                sc.wait_ge(s_topk, g + 1)
                sc.activation(
                    out=u_f[g][:], in_=idxu[g][:, 0:_NPATCH],
                    func=Act.Identity, scale=0.14285715, bias=bias07[:])
                sc.activation(
                    out=br_i[g][:], in_=u_f[g][:], func=Act.Identity)
                sc.activation(
                    out=br_f[g][:], in_=br_i[g][:], func=Act.Identity)
                sc.activation(
                    out=idx_f[g][:], in_=idxu[g][:, 0:_NPATCH],
                    func=Act.Identity)
                sc.drain().then_inc(s_brf, 1)
                # cast psD (PSUM, exact integers) + 7*(p%16) -> idx16 slice
                # (i16, SBUF); gpsimd cannot read PSUM, the Act engine can
                sc.wait_ge(s_bmm, g + 1)
                sl = slice(168 * g, 168 * g + 168)
                sc.activation(
                    out=idx16[:, sl], in_=psD[g][:], func=Act.Identity,
                    scale=1.0, bias=bias7w[:])
                sc.drain().then_inc(s_psd, 1)
            for g in (1, 3):
                sc.wait_ge(s_gc[g], 64)
                sc.dma_start(
                    out=out_pc[:, 21 * g:21 * g + 21, :],
                    in_=GT[:, 21 * g:21 * g + 21, :],
                ).then_inc(s_st, 16)


        @block.vector
        def _(vector):
            # constants (disjoint writes, no deps)
            vector.memset(G64[0:64, 0:1], 1.0)
            vector.memset(G64[0:64, 1:2], 0.0)
            vector.memset(G64[64:128, 0:1], 0.0)
            vector.memset(G64[64:128, 1:2], 1.0)
            vector.memset(ones128[:], 1.0)
            for b_ in range(2):
                vector.memset(sel1[b_][:, :], 0.0)
                vector.memset(sel1[b_][32 * b_:32 * b_ + 1, :], 1.0)
                vector.memset(sel441[b_][:, :], 0.0)
                vector.memset(sel441[b_][32 * b_:32 * b_ + 1, :], 441.0)
            vector.memset(warmidx[:], 0)
            # DVE CAST f32->i32 rounds to nearest: floor(x) == round(x - 0.5)
            # u = (idx + 0.5)/7 - 0.5 keeps >=0.07 margin from half-integers
            vector.memset(bias07[:], 0.071428575 - 0.5)
            vector.drain().then_inc(s_ones, 1)

            def R_stage(g):
                if g == 0:
                    vector.wait_ge(s_loadE, 16)
                    vector.reduce_sum(
                        out=red[0][:, 0:4, :],
                        in_=hmB[0][:, 0:4, :].rearrange(
                            "p rhi (bc u) -> p rhi bc u", u=64),
                        axis=X,
                    )
                    vector.wait_ge(s_loadE, 32)
                    vector.reduce_sum(
                        out=red[0][:, 4:7, :],
                        in_=hmB[0][:, 4:7, :].rearrange(
                            "p rhi (bc u) -> p rhi bc u", u=64),
                        axis=X,
                    )
                    vector.drain().then_inc(s_red, 1)
                    return
                if g == 2:
                    vector.wait_ge(s_loadE, 48)
                else:
                    vector.wait_ge(s_loadO, 16 * (g // 2 + 1))
                vector.reduce_sum(
                    out=red[g][:],
                    in_=hmB[g][:].rearrange("p rhi (bc u) -> p rhi bc u",
                                            u=64),
                    axis=X,
                )
                vector.drain().then_inc(s_red, 1)

            def T_stage(g):
                # two top-24 dances (one per batch) on [1,49] slices of
                # psV_g, rounds interleaved to hide drain latency
                vector.wait_ge(s_pmm, g + 1)
                vector.tensor_copy(
                    out=Vt[:].rearrange("p (br bc) -> p br bc", br=7),
                    in_=(psV[g][:]
                         .rearrange("p (br bc8) -> p br bc8",
                                    br=8)[:, 0:7, 0:7]),
                )
                vector.drain()
                cur = Vt
                for r3 in range(3):
                    vector.max(out=m2[:], in_=cur[:])
                    vector.drain()
                    vector.max_index(
                        out=idxu[g][:, 8 * r3:8 * r3 + 8], in_max=m2[:],
                        in_values=cur[:])
                    if r3 < 2:
                        nxt = vw[r3]
                        vector.match_replace(
                            out=nxt[:], in_to_replace=m2[:], in_values=cur[:],
                            imm_value=-1e30)
                        vector.drain()
                        cur = nxt
                vector.drain().then_inc(s_topk, 1)

            # reduce one block ahead so each P matmul (and its semaphore
            # hop) overlaps the next reduce instead of sitting between
            # R_g and T_g on the critical path
            R_stage(0)
            R_stage(1)
            T_stage(0)
            R_stage(2)
            T_stage(1)
            R_stage(3)
            T_stage(2)
            T_stage(3)

        @block.tensor
        def _(tensor):
            def P_stage(g):
                # interleaved rows: pooled group q = 2n + p//64 = 7b + br.
                # Four masked matmuls (per batch b x partition-half g2)
                # write psV2[g] [1,128] at 64b + 8*br + bc (pad unread).
                tensor.wait_ge(s_red, g + 1)
                if g == 0:
                    tensor.wait_ge(s_ones, 1)
                pieces = [
                    (0, 0, 0, 4, 0),   # b, g2, n0, cnt, two (br = 2n+g2-7b)
                    (0, 1, 0, 3, 1),
                    (1, 0, 4, 3, 1),
                    (1, 1, 3, 4, 0),
                ]
                for i, (b, g2, n0, cnt, two) in enumerate(pieces):
                    hb = (psV[g][32 * b:32 * b + 1, :]
                          .rearrange("p (n two bc8) -> p n two bc8",
                                     n=4, two=2, bc8=8))
                    tensor.matmul(
                        out=hb[:, 0:cnt, two:two + 1, 0:7],
                        lhsT=G64[:, g2:g2 + 1],
                        rhs=(red[g][:, n0:n0 + cnt, :]
                             .rearrange("p n (one bc) -> p n one bc",
                                        one=1)),
                        start=True, stop=True,
                    ).then_maybe_inc((s_pmm, 1) if i == 3 else None)

            def B_stage(g):
                # psD[g] = idx + 441*br (x4 along free) + static s-term
                tensor.wait_ge(s_brf, g + 1)
                if g == 0:
                    tensor.wait_ge(s_stt, 32)
                ridx = (idx_f[g][:]
                        .rearrange("p (m one) -> p m one", one=1)
                        .to_broadcast([33, _NPATCH, 4]))
                rbr = (br_f[g][:]
                       .rearrange("p (m one) -> p m one", one=1)
                       .to_broadcast([33, _NPATCH, 4]))
                for b in range(2):
                    sl = slice(84 * b, 84 * b + 84)
                    m_ = 2 * g + b
                    tensor.matmul(
                        out=psD[g][:, sl], lhsT=sel1[b][:], rhs=ridx,
                        start=True, stop=False)
                    tensor.matmul(
                        out=psD[g][:, sl], lhsT=sel441[b][:], rhs=rbr,
                        start=False, stop=False)
                    tensor.matmul(
                        out=psD[g][:, sl], lhsT=ones128[:],
                        rhs=statc[0:1, 84 * m_:84 * m_ + 84],
                        start=False, stop=True,
                    ).then_maybe_inc((s_bmm, 1) if b == 1 else None)

            P_stage(0)
            B_stage(0)
            P_stage(1)
            B_stage(1)
            P_stage(2)
            B_stage(2)
            P_stage(3)
            B_stage(3)

        @block.gpsimd
        def _(g):
            # preload the extended-instruction library early so the ucode
            # overlay DMA overlaps the heatmap phase
            from concourse import library_config
            g.load_library(library_config.mlp)
            # dummy gather absorbs any one-time ucode init cost
            g.wait_ge(s_ones, 1)
            g.dma_gather(
                out_ap=GTwarm[:],
                in_ap=img_rows,
                idxs_ap=warmidx[:],
                num_idxs=128,
                num_idxs_reg=128,
                elem_size=_PROW,
                queue_num=0,
            ).then_inc(s_warm, 16)
            g.wait_ge(s_warm, 16)
            # 4 calls per block on queues 0-3 so all four SWDGE core-pairs
            # expand descriptors concurrently
            for blk in range(_NBLK):
                g.wait_ge(s_psd, blk + 1)
                for c in range(4):
                    n = 128 * _CALL_COLS[c]
                    lo = 21 * blk + _CALL_OFF[c]
                    ilo = 168 * blk + 8 * _CALL_OFF[c]
                    g.dma_gather(
                        out_ap=GT[:, lo:lo + _CALL_COLS[c], :],
                        in_ap=img_rows,
                        idxs_ap=idx16[:, ilo:ilo + 8 * _CALL_COLS[c]],
                        num_idxs=n,
                        num_idxs_reg=n,
                        elem_size=_PROW,
                        queue_num=c,
                    ).then_inc(s_gc[blk], 16)

    nc.finalize()
    _nc_cache = nc
    return nc


def kernel(heatmap, image):
    from concourse.bass_utils import run_bass_kernel_spmd

    heatmap = np.ascontiguousarray(np.asarray(heatmap), dtype=np.float32)
    image = np.ascontiguousarray(np.asarray(image), dtype=np.float32)
    assert heatmap.shape == (_B, 448, 448, 1)
    assert image.shape == (_B, 448, 448, 3)

    nc = build_program()
    in_maps = [
        {
            "heatmap": heatmap[c * _B_LOC:(c + 1) * _B_LOC],
            "image": image[c * _B_LOC:(c + 1) * _B_LOC],
        }
        for c in range(_N_CORES)
    ]
    res = run_bass_kernel_spmd(nc, in_maps, list(range(_N_CORES)))
    outs = [res.results[c]["out"] for c in range(_N_CORES)]
    return np.concatenate(outs, axis=0)


# revision 55
# speedup vs baseline: 1.0291x; 1.0291x over previous
"""Trainium2 Bass kernel for ExtractRelevantPatches (pool -> top-k -> gather).

Full-input contract: kernel(heatmap [64,448,448,1] f32, image [64,448,448,3] f32)
-> [1344, 64, 64, 3] f32.

Sharding: pure data-parallel over batch; 8 batches per NeuronCore, 8 cores.

Pipelined per-core algorithm (raw Bass, explicit semaphores), 4 blocks of
2 batches, so gather descriptor expansion + HBM flow of early blocks overlap
pool/top-k of later blocks:

  Per block g (batches 2g, 2g+1):
  1. L_g: batch 2g on the sync HWDGE queue -> SBUF partitions 0-63, batch
     2g+1 on the scalar HWDGE queue -> partitions 64-127 (concurrent
     64-partition streams; SBUF write bw is ~2.9 GB/s/partition).
  2. R_g (DVE): reduce_sum over 64-col groups -> red_g [128, 7, 7].
  3. P_g (TensorE): two single-column matmuls with the 0/1 matrix G64
     (G64[p,b]=1 iff p//64==b) -> psV_g [1, 98]: BOTH batches' 49 pooled
     sums side by side on partition 0.  Ranking by sums == by means.
  4. T_g (DVE): per-batch top-24 on [1,49] slices, the two batches' rounds
     interleaved (max8 / find_index8 / match_replace8 x3); keep first 21
     (descending, matching jax top_k).  idx_f [1, 42] = cast of both idx
     lists, b-major -- already flat on partition 0, so no cross-partition
     flatten DMA is needed anywhere.
  5. A_g (Act): u = (idx + 0.5)*(1/7) - 0.5 (scale+bias in one activation).
  6. C_g (DVE): br = cast_i32(u) (round-to-nearest == floor after the -0.5
     bias); br_f = cast_f32(br).  No DVE integer multiplies anywhere (int
     TENSOR_SCALAR mult stalls ~7.5us while gpsimd descriptor expansion
     runs).
  7. B_g (TensorE): three accumulating broadcast matmuls:
     psD_g[128, 168] = 1*idx_f + 441*br_f (x4 along free via to_broadcast)
     + statc row (the s-dependent static part 112*(s%4) + 3136*b).
  8. cp_g (Act): idx16 slice = cast_i16(psD_g + 7*(p%16)) via bias AP.
     gpsimd cannot read PSUM; the Act engine can.
  9. G_g (gpsimd SWDGE): 4 dma_gather calls (768/768/640/512 idxs) on
     queues 0-3 -> all four SWDGE core-pairs expand descriptors
     concurrently (~9ns/desc per pair; the pipeline's pacing resource).
 10. S_gc: per-call stores GT cols -> DRAM out on the sync HWDGE queue.
"""

import numpy as np

_N_CORES = 8
_B = 64
_B_LOC = _B // _N_CORES  # 8
_PATCH = 64
_GRID = 7
_NPATCH = 21
_PROW = _PATCH * 3            # 192 elements per patch-row
_OUT_ROWS_LOC = _B_LOC * _NPATCH  # 168
_NBLK = 4                     # blocks of 2 batches
_CALL_COLS = [6, 6, 5, 4]     # gather call sizes per block, in 128-row cols
_CALL_OFF = [0, 6, 12, 17]

_nc_cache = None


def build_program():
    """Build the per-core SPMD Bass program (cached)."""
    global _nc_cache
    if _nc_cache is not None:
        return _nc_cache

    import concourse.bass as bass
    import concourse.bacc as bacc
    import concourse.mybir as mybir

    f32 = mybir.dt.float32
    i16 = mybir.dt.int16
    i32 = mybir.dt.int32
    u32 = mybir.dt.uint32
    X = mybir.AxisListType.X
    Op = mybir.AluOpType
    Act = mybir.ActivationFunctionType

    nc = bacc.Bacc(num_swdge_queues=4)

    hm_in = nc.declare_dram_parameter(
        "heatmap", [_B_LOC, 448, 448, 1], f32, isOutput=False)
    img_in = nc.declare_dram_parameter(
        "image", [_B_LOC, 448, 448, 3], f32, isOutput=False)
    out_t = nc.declare_dram_parameter(
        "out", [_OUT_ROWS_LOC, _PATCH, _PATCH, 3], f32, isOutput=True)

    # Static parts of the gather index list: position i = R at [R%16, R//16],
    # R = 16*s + w; static term = 112*(s%4) + 7*w + 3136*(R//1344).
    # The s-dependent part rides the third accumulating matmul (statc row
    # per block); the w-dependent 7*(p%16) part rides the Act copy's bias.
    s_ar = np.arange(84, dtype=np.int64)
    stat = np.zeros((1, 672), dtype=np.float32)
    for m_ in range(8):
        stat[0, 84 * m_:84 * m_ + 84] = 112 * (s_ar % 4) + 3136 * m_
    statc_const = nc.inline_tensor(stat, name="statc_const")
    b7 = (7.0 * (np.arange(128) % 16)).reshape(128, 1).astype(np.float32)
    bias7w_const = nc.inline_tensor(b7, name="bias7w_const")

    # heatmap block view [128, 4, 7, 448]: within block g, row R2 = 448*bp+r
    # sits at partition R2%128, free (R2//128, col) -- one full-rate
    # 128-partition DMA per 2-batch block
    hm_blk = (hm_in[:]
              .rearrange("(g two) r c one -> g (two r) (c one)", two=2)
              .rearrange("g (n p) c -> p g n c", p=128))

    # image patch-row view: [25088, 192]
    img_rows = (img_in[:]
                .rearrange("b r c ch -> (b r c ch)")
                .rearrange("(n e) -> n e", e=_PROW))

    # output patch-row view [10752, 192] -> [p, c, e] with R = 128*c + p
    out_pc = (out_t[:]
              .rearrange("r a b c -> (r a b c)")
              .rearrange("(n e) -> n e", e=_PROW)
              .rearrange("(c p) e -> p c e", p=128))

    from contextlib import ExitStack

    with ExitStack() as ctx:
        e = ctx.enter_context
        hmB = [e(nc.sbuf_tensor(f"hm{g}", [128, 7, 448], f32))
               for g in range(_NBLK)]
        red = [e(nc.sbuf_tensor(f"red{g}", [128, 7, 7], f32))
               for g in range(_NBLK)]
        G64 = e(nc.sbuf_tensor("G64", [128, 2], f32))
        ones128 = e(nc.sbuf_tensor("ones128", [1, 128], f32))
        sel1 = [e(nc.sbuf_tensor(f"sel1_{b}", [33, 128], f32))
                for b in range(2)]
        sel441 = [e(nc.sbuf_tensor(f"sel441_{b}", [33, 128], f32))
                  for b in range(2)]
        Vt = e(nc.sbuf_tensor("Vt", [33, 49], f32))
        vw = [e(nc.sbuf_tensor(f"vw{i}", [33, 49], f32)) for i in range(2)]
        m2 = e(nc.sbuf_tensor("m2", [33, 8], f32))
        idxu = [e(nc.sbuf_tensor(f"idxu{g}", [33, 24], u32))
                for g in range(_NBLK)]
        idx_f = [e(nc.sbuf_tensor(f"idx_f{g}", [33, _NPATCH], f32))
                 for g in range(_NBLK)]
        u_f = [e(nc.sbuf_tensor(f"u_f{g}", [33, _NPATCH], f32))
               for g in range(_NBLK)]
        br_i = [e(nc.sbuf_tensor(f"br_i{g}", [33, _NPATCH], i32))
                for g in range(_NBLK)]
        br_f = [e(nc.sbuf_tensor(f"br_f{g}", [33, _NPATCH], f32))
                for g in range(_NBLK)]
        bias07 = e(nc.sbuf_tensor("bias07", [33, 1], f32))
        statc = e(nc.sbuf_tensor("statc", [1, 672], f32))
        bias7w = e(nc.sbuf_tensor("bias7w", [128, 1], f32))
        idx16 = e(nc.sbuf_tensor("idx16", [128, 672], i16))
        warmidx = e(nc.sbuf_tensor("warmidx", [128, 8], i16))
        GT = e(nc.sbuf_tensor("GT", [128, 84, _PROW], f32))
        GTwarm = e(nc.sbuf_tensor("GTwarm", [128, 1, _PROW], f32))
        psV = [e(nc.psum_tensor(f"psV{g}", [33, 64], f32))
               for g in range(_NBLK)]
        psD = [e(nc.psum_tensor(f"psD{g}", [128, 168], f32))
               for g in range(_NBLK)]
        s_loadE = e(nc.semaphore("s_loadE"))
        s_loadO = e(nc.semaphore("s_loadO"))
        s_stt = e(nc.semaphore("s_stt"))
        s_red = e(nc.semaphore("s_red"))
        s_pmm = e(nc.semaphore("s_pmm"))
        s_topk = e(nc.semaphore("s_topk"))
        s_act = e(nc.semaphore("s_act"))
        s_brf = e(nc.semaphore("s_brf"))
        s_psd = e(nc.semaphore("s_psd"))
        s_bmm = e(nc.semaphore("s_bmm"))
        s_ones = e(nc.semaphore("s_ones"))
        s_warm = e(nc.semaphore("s_warm"))
        s_gc = [e(nc.semaphore(f"s_gc{k}")) for k in range(_NBLK)]
        s_st = e(nc.semaphore("s_st"))
        block = e(nc.Block())

        @block.sync
        def _(sync):
            for g in (0, 2):
                sync.dma_start(
                    out=hmB[g][:].rearrange("p (one rhi) c -> p one rhi c",
                                            one=1),
                    in_=hm_blk[:, g:g + 1, :, :],
                ).then_inc(s_loadE, 16)
            for g in (0, 2):
                sync.wait_ge(s_gc[g], 64)
                sync.dma_start(
                    out=out_pc[:, 21 * g:21 * g + 21, :],
                    in_=GT[:, 21 * g:21 * g + 21, :],
                ).then_inc(s_st, 16)
            sync.wait_ge(s_st, 64)

        @block.scalar
        def _(sc):
            sc.dma_start(
                out=hmB[1][:].rearrange("p (one rhi) c -> p one rhi c",
                                        one=1),
                in_=hm_blk[:, 1:2, :, :],
            ).then_inc(s_loadO, 16)
            sc.dma_start(out=statc[:], in_=statc_const[:]).then_inc(s_stt, 16)
            sc.dma_start(
                out=bias7w[:], in_=bias7w_const[:]).then_inc(s_stt, 16)
            sc.dma_start(
                out=hmB[3][:].rearrange("p (one rhi) c -> p one rhi c",
                                        one=1),
                in_=hm_blk[:, 3:4, :, :],
            ).then_inc(s_loadO, 16)
            for g in range(_NBLK):
                # DVE dtype-converting ops stall behind gpsimd descriptor
                # expansion, so ALL casts live on the Act engine:
                # idx_f = cast_f32(idxu); u = (idx+0.5)/7 - 0.5;
                # br = cast_i32(u) (round == floor after the -0.5 bias);
                # br_f = cast_f32(br)
                sc.wait_ge(s_topk, g + 1)
                sc.activation(
                    out=u_f[g][:], in_=idxu[g][:, 0:_NPATCH],
                    func=Act.Identity, scale=0.14285715, bias=bias07[:])
                sc.activation(
                    out=br_i[g][:], in_=u_f[g][:], func=Act.Identity)
                sc.activation(
                    out=br_f[g][:], in_=br_i[g][:], func=Act.Identity)
                sc.activation(
                    out=idx_f[g][:], in_=idxu[g][:, 0:_NPATCH],
                    func=Act.Identity)
                sc.drain().then_inc(s_brf, 1)
                # cast psD (PSUM, exact integers) + 7*(p%16) -> idx16 slice
                # (i16, SBUF); gpsimd cannot read PSUM, the Act engine can
                sc.wait_ge(s_bmm, g + 1)
                sl = slice(168 * g, 168 * g + 168)
                sc.activation(
                    out=idx16[:, sl], in_=psD[g][:], func=Act.Identity,
                    scale=1.0, bias=bias7w[:])
                sc.drain().then_inc(s_psd, 1)
            for g in (1, 3):
                sc.wait_ge(s_gc[g], 64)
                sc.dma_start(
                    out=out_pc[:, 21 * g:21 * g + 21, :],
                    in_=GT[:, 21 * g:21 * g + 21, :],
                ).then_inc(s_st, 16)


        @block.vector
        def _(vector):
            # constants (disjoint writes, no deps)
            vector.memset(G64[0:64, 0:1], 1.0)
            vector.memset(G64[0:64, 1:2], 0.0)
            vector.memset(G64[64:128, 0:1], 0.0)
            vector.memset(G64[64:128, 1:2], 1.0)
            vector.memset(ones128[:], 1.0)
            for b_ in range(2):
                vector.memset(sel1[b_][:, :], 0.0)
                vector.memset(sel1[b_][32 * b_:32 * b_ + 1, :], 1.0)
                vector.memset(sel441[b_][:, :], 0.0)
                vector.memset(sel441[b_][32 * b_:32 * b_ + 1, :], 441.0)
            vector.memset(warmidx[:], 0)
            # DVE CAST f32->i32 rounds to nearest: floor(x) == round(x - 0.5)
            # u = (idx + 0.5)/7 - 0.5 keeps >=0.07 margin from half-integers
            vector.memset(bias07[:], 0.071428575 - 0.5)
            vector.drain().then_inc(s_ones, 1)

            def R_stage(g):
                if g % 2 == 0:
                    vector.wait_ge(s_loadE, 16 * (g // 2 + 1))
                else:
                    vector.wait_ge(s_loadO, 16 * (g // 2 + 1))
                vector.reduce_sum(
                    out=red[g][:],
                    in_=hmB[g][:].rearrange("p rhi (bc u) -> p rhi bc u",
                                            u=64),
                    axis=X,
                )
                vector.drain().then_inc(s_red, 1)

            def T_stage(g):
                # two top-24 dances (one per batch) on [1,49] slices of
                # psV_g, rounds interleaved to hide drain latency
                vector.wait_ge(s_pmm, g + 1)
                vector.tensor_copy(
                    out=Vt[:].rearrange("p (br bc) -> p br bc", br=7),
                    in_=(psV[g][:]
                         .rearrange("p (br bc8) -> p br bc8",
                                    br=8)[:, 0:7, 0:7]),
                )
                vector.drain()
                cur = Vt
                for r3 in range(3):
                    vector.max(out=m2[:], in_=cur[:])
                    vector.drain()
                    vector.max_index(
                        out=idxu[g][:, 8 * r3:8 * r3 + 8], in_max=m2[:],
                        in_values=cur[:])
                    if r3 < 2:
                        nxt = vw[r3]
                        vector.match_replace(
                            out=nxt[:], in_to_replace=m2[:], in_values=cur[:],
                            imm_value=-1e30)
                        vector.drain()
                        cur = nxt
                vector.drain().then_inc(s_topk, 1)

            # reduce one block ahead so each P matmul (and its semaphore
            # hop) overlaps the next reduce instead of sitting between
            # R_g and T_g on the critical path
            R_stage(0)
            R_stage(1)
            T_stage(0)
            R_stage(2)
            T_stage(1)
            R_stage(3)
            T_stage(2)
            T_stage(3)

        @block.tensor
        def _(tensor):
            def P_stage(g):
                # interleaved rows: pooled group q = 2n + p//64 = 7b + br.
                # Four masked matmuls (per batch b x partition-half g2)
                # write psV2[g] [1,128] at 64b + 8*br + bc (pad unread).
                tensor.wait_ge(s_red, g + 1)
                if g == 0:
                    tensor.wait_ge(s_ones, 1)
                pieces = [
                    (0, 0, 0, 4, 0),   # b, g2, n0, cnt, two (br = 2n+g2-7b)
                    (0, 1, 0, 3, 1),
                    (1, 0, 4, 3, 1),
                    (1, 1, 3, 4, 0),
                ]
                for i, (b, g2, n0, cnt, two) in enumerate(pieces):
                    hb = (psV[g][32 * b:32 * b + 1, :]
                          .rearrange("p (n two bc8) -> p n two bc8",
                                     n=4, two=2, bc8=8))
                    tensor.matmul(
                        out=hb[:, 0:cnt, two:two + 1, 0:7],
                        lhsT=G64[:, g2:g2 + 1],
                        rhs=(red[g][:, n0:n0 + cnt, :]
                             .rearrange("p n (one bc) -> p n one bc",
                                        one=1)),
                        start=True, stop=True,
                    ).then_maybe_inc((s_pmm, 1) if i == 3 else None)

            def B_stage(g):
                # psD[g] = idx + 441*br (x4 along free) + static s-term
                tensor.wait_ge(s_brf, g + 1)
                if g == 0:
                    tensor.wait_ge(s_stt, 32)
                ridx = (idx_f[g][:]
                        .rearrange("p (m one) -> p m one", one=1)
                        .to_broadcast([33, _NPATCH, 4]))
                rbr = (br_f[g][:]
                       .rearrange("p (m one) -> p m one", one=1)
                       .to_broadcast([33, _NPATCH, 4]))
                for b in range(2):
                    sl = slice(84 * b, 84 * b + 84)
                    m_ = 2 * g + b
                    tensor.matmul(
                        out=psD[g][:, sl], lhsT=sel1[b][:], rhs=ridx,
                        start=True, stop=False)
                    tensor.matmul(
                        out=psD[g][:, sl], lhsT=sel441[b][:], rhs=rbr,
                        start=False, stop=False)
                    tensor.matmul(
                        out=psD[g][:, sl], lhsT=ones128[:],
                        rhs=statc[0:1, 84 * m_:84 * m_ + 84],
                        start=False, stop=True,
                    ).then_maybe_inc((s_bmm, 1) if b == 1 else None)

            P_stage(0)
            B_stage(0)
            P_stage(1)
            B_stage(1)
            P_stage(2)
            B_stage(2)
            P_stage(3)
            B_stage(3)

        @block.gpsimd
        def _(g):
            # preload the extended-instruction library early so the ucode
            # overlay DMA overlaps the heatmap phase
            from concourse import library_config
            g.load_library(library_config.mlp)
            # dummy gather absorbs any one-time ucode init cost
            g.wait_ge(s_ones, 1)
            g.dma_gather(
                out_ap=GTwarm[:],
                in_ap=img_rows,
                idxs_ap=warmidx[:],
                num_idxs=128,
                num_idxs_reg=128,
                elem_size=_PROW,
                queue_num=0,
            ).then_inc(s_warm, 16)
            g.wait_ge(s_warm, 16)
            # 4 calls per block on queues 0-3 so all four SWDGE core-pairs
            # expand descriptors concurrently
            for blk in range(_NBLK):
                g.wait_ge(s_psd, blk + 1)
                for c in range(4):
                    n = 128 * _CALL_COLS[c]
                    lo = 21 * blk + _CALL_OFF[c]
                    ilo = 168 * blk + 8 * _CALL_OFF[c]
                    g.dma_gather(
                        out_ap=GT[:, lo:lo + _CALL_COLS[c], :],
                        in_ap=img_rows,
                        idxs_ap=idx16[:, ilo:ilo + 8 * _CALL_COLS[c]],
                        num_idxs=n,
                        num_idxs_reg=n,
                        elem_size=_PROW,
                        queue_num=c,
                    ).then_inc(s_gc[blk], 16)

    nc.finalize()
    _nc_cache = nc
    return nc


def kernel(heatmap, image):
    from concourse.bass_utils import run_bass_kernel_spmd

    heatmap = np.ascontiguousarray(np.asarray(heatmap), dtype=np.float32)
    image = np.ascontiguousarray(np.asarray(image), dtype=np.float32)
    assert heatmap.shape == (_B, 448, 448, 1)
    assert image.shape == (_B, 448, 448, 3)

    nc = build_program()
    in_maps = [
        {
            "heatmap": heatmap[c * _B_LOC:(c + 1) * _B_LOC],
            "image": image[c * _B_LOC:(c + 1) * _B_LOC],
        }
        for c in range(_N_CORES)
    ]
    res = run_bass_kernel_spmd(nc, in_maps, list(range(_N_CORES)))
    outs = [res.results[c]["out"] for c in range(_N_CORES)]
    return np.concatenate(outs, axis=0)


# revision 56
# speedup vs baseline: 1.0858x; 1.0552x over previous
"""Trainium2 Bass kernel for ExtractRelevantPatches (pool -> top-k -> gather).

Full-input contract: kernel(heatmap [64,448,448,1] f32, image [64,448,448,3] f32)
-> [1344, 64, 64, 3] f32.

Sharding: pure data-parallel over batch; 8 batches per NeuronCore, 8 cores.

Pipelined per-core algorithm (raw Bass, explicit semaphores), 4 blocks of
2 batches, so gather descriptor expansion + HBM flow of early blocks overlap
pool/top-k of later blocks:

  Per block g (batches 2g, 2g+1):
  1. L_g: batch 2g on the sync HWDGE queue -> SBUF partitions 0-63, batch
     2g+1 on the scalar HWDGE queue -> partitions 64-127 (concurrent
     64-partition streams; SBUF write bw is ~2.9 GB/s/partition).
  2. R_g (DVE): reduce_sum over 64-col groups -> red_g [128, 7, 7].
  3. P_g (TensorE): two single-column matmuls with the 0/1 matrix G64
     (G64[p,b]=1 iff p//64==b) -> psV_g [1, 98]: BOTH batches' 49 pooled
     sums side by side on partition 0.  Ranking by sums == by means.
  4. T_g (DVE): per-batch top-24 on [1,49] slices, the two batches' rounds
     interleaved (max8 / find_index8 / match_replace8 x3); keep first 21
     (descending, matching jax top_k).  idx_f [1, 42] = cast of both idx
     lists, b-major -- already flat on partition 0, so no cross-partition
     flatten DMA is needed anywhere.
  5. A_g (Act): u = (idx + 0.5)*(1/7) - 0.5 (scale+bias in one activation).
  6. C_g (DVE): br = cast_i32(u) (round-to-nearest == floor after the -0.5
     bias); br_f = cast_f32(br).  No DVE integer multiplies anywhere (int
     TENSOR_SCALAR mult stalls ~7.5us while gpsimd descriptor expansion
     runs).
  7. B_g (TensorE): three accumulating broadcast matmuls:
     psD_g[128, 168] = 1*idx_f + 441*br_f (x4 along free via to_broadcast)
     + statc row (the s-dependent static part 112*(s%4) + 3136*b).
  8. cp_g (Act): idx16 slice = cast_i16(psD_g + 7*(p%16)) via bias AP.
     gpsimd cannot read PSUM; the Act engine can.
  9. G_g (gpsimd SWDGE): 4 dma_gather calls (768/768/640/512 idxs) on
     queues 0-3 -> all four SWDGE core-pairs expand descriptors
     concurrently (~9ns/desc per pair; the pipeline's pacing resource).
 10. S_gc: per-call stores GT cols -> DRAM out on the sync HWDGE queue.
"""

import numpy as np

_N_CORES = 8
_B = 64
_B_LOC = _B // _N_CORES  # 8
_PATCH = 64
_GRID = 7
_NPATCH = 21
_PROW = _PATCH * 3            # 192 elements per patch-row
_OUT_ROWS_LOC = _B_LOC * _NPATCH  # 168
_NBLK = 4                     # blocks of 2 batches
_CALL_COLS = [6, 5, 5, 5]     # gather call sizes per block, in 128-row cols
_CALL_OFF = [0, 6, 11, 16]

_nc_cache = None


def build_program():
    """Build the per-core SPMD Bass program (cached)."""
    global _nc_cache
    if _nc_cache is not None:
        return _nc_cache

    import concourse.bass as bass
    import concourse.bacc as bacc
    import concourse.mybir as mybir

    f32 = mybir.dt.float32
    i16 = mybir.dt.int16
    i32 = mybir.dt.int32
    u32 = mybir.dt.uint32
    X = mybir.AxisListType.X
    Op = mybir.AluOpType
    Act = mybir.ActivationFunctionType

    nc = bacc.Bacc(num_swdge_queues=4)

    hm_in = nc.declare_dram_parameter(
        "heatmap", [_B_LOC, 448, 448, 1], f32, isOutput=False)
    img_in = nc.declare_dram_parameter(
        "image", [_B_LOC, 448, 448, 3], f32, isOutput=False)
    out_t = nc.declare_dram_parameter(
        "out", [_OUT_ROWS_LOC, _PATCH, _PATCH, 3], f32, isOutput=True)

    # Static parts of the gather index list: position i = R at [R%16, R//16],
    # R = 16*s + w; static term = 112*(s%4) + 7*w + 3136*(R//1344).
    # The s-dependent part rides the third accumulating matmul (statc row
    # per block); the w-dependent 7*(p%16) part rides the Act copy's bias.
    s_ar = np.arange(84, dtype=np.int64)
    stat = np.zeros((1, 672), dtype=np.float32)
    for m_ in range(8):
        stat[0, 84 * m_:84 * m_ + 84] = 112 * (s_ar % 4) + 3136 * m_
    statc_const = nc.inline_tensor(stat, name="statc_const")
    b7 = (7.0 * (np.arange(128) % 16)).reshape(128, 1).astype(np.float32)
    bias7w_const = nc.inline_tensor(b7, name="bias7w_const")

    # heatmap block view [128, 4, 7, 448]: within block g, row R2 = 448*bp+r
    # sits at partition R2%128, free (R2//128, col) -- one full-rate
    # 128-partition DMA per 2-batch block
    hm_blk = (hm_in[:]
              .rearrange("(g two) r c one -> g (two r) (c one)", two=2)
              .rearrange("g (n p) c -> p g n c", p=128))

    # image patch-row view: [25088, 192]
    img_rows = (img_in[:]
                .rearrange("b r c ch -> (b r c ch)")
                .rearrange("(n e) -> n e", e=_PROW))

    # output patch-row view [10752, 192] -> [p, c, e] with R = 128*c + p
    out_pc = (out_t[:]
              .rearrange("r a b c -> (r a b c)")
              .rearrange("(n e) -> n e", e=_PROW)
              .rearrange("(c p) e -> p c e", p=128))

    from contextlib import ExitStack

    with ExitStack() as ctx:
        e = ctx.enter_context
        hmB = [e(nc.sbuf_tensor(f"hm{g}", [128, 7, 448], f32))
               for g in range(_NBLK)]
        red = [e(nc.sbuf_tensor(f"red{g}", [128, 7, 7], f32))
               for g in range(_NBLK)]
        G64 = e(nc.sbuf_tensor("G64", [128, 2], f32))
        ones128 = e(nc.sbuf_tensor("ones128", [1, 128], f32))
        sel1 = [e(nc.sbuf_tensor(f"sel1_{b}", [33, 128], f32))
                for b in range(2)]
        sel441 = [e(nc.sbuf_tensor(f"sel441_{b}", [33, 128], f32))
                  for b in range(2)]
        Vt = e(nc.sbuf_tensor("Vt", [33, 49], f32))
        vw = [e(nc.sbuf_tensor(f"vw{i}", [33, 49], f32)) for i in range(2)]
        m2 = e(nc.sbuf_tensor("m2", [33, 8], f32))
        idxu = [e(nc.sbuf_tensor(f"idxu{g}", [33, 24], u32))
                for g in range(_NBLK)]
        idx_f = [e(nc.sbuf_tensor(f"idx_f{g}", [33, _NPATCH], f32))
                 for g in range(_NBLK)]
        u_f = [e(nc.sbuf_tensor(f"u_f{g}", [33, _NPATCH], f32))
               for g in range(_NBLK)]
        br_i = [e(nc.sbuf_tensor(f"br_i{g}", [33, _NPATCH], i32))
                for g in range(_NBLK)]
        br_f = [e(nc.sbuf_tensor(f"br_f{g}", [33, _NPATCH], f32))
                for g in range(_NBLK)]
        bias07 = e(nc.sbuf_tensor("bias07", [33, 1], f32))
        statc = e(nc.sbuf_tensor("statc", [1, 672], f32))
        bias7w = e(nc.sbuf_tensor("bias7w", [128, 1], f32))
        idx16 = e(nc.sbuf_tensor("idx16", [128, 672], i16))
        warmidx = e(nc.sbuf_tensor("warmidx", [128, 8], i16))
        GT = e(nc.sbuf_tensor("GT", [128, 84, _PROW], f32))
        GTwarm = e(nc.sbuf_tensor("GTwarm", [128, 1, _PROW], f32))
        psV = [e(nc.psum_tensor(f"psV{g}", [33, 64], f32))
               for g in range(_NBLK)]
        psD = [e(nc.psum_tensor(f"psD{g}", [128, 168], f32))
               for g in range(_NBLK)]
        s_loadE = e(nc.semaphore("s_loadE"))
        s_loadO = e(nc.semaphore("s_loadO"))
        s_stt = e(nc.semaphore("s_stt"))
        s_red = e(nc.semaphore("s_red"))
        s_pmm = e(nc.semaphore("s_pmm"))
        s_topk = e(nc.semaphore("s_topk"))
        s_act = e(nc.semaphore("s_act"))
        s_brf = e(nc.semaphore("s_brf"))
        s_psd = e(nc.semaphore("s_psd"))
        s_bmm = e(nc.semaphore("s_bmm"))
        s_ones = e(nc.semaphore("s_ones"))
        s_warm = e(nc.semaphore("s_warm"))
        s_gc = [e(nc.semaphore(f"s_gc{k}")) for k in range(_NBLK)]
        s_st = e(nc.semaphore("s_st"))
        block = e(nc.Block())

        @block.sync
        def _(sync):
            for g in (0, 2):
                sync.dma_start(
                    out=hmB[g][:].rearrange("p (one rhi) c -> p one rhi c",
                                            one=1),
                    in_=hm_blk[:, g:g + 1, :, :],
                ).then_inc(s_loadE, 16)
            for g in (0, 2):
                sync.wait_ge(s_gc[g], 64)
                sync.dma_start(
                    out=out_pc[:, 21 * g:21 * g + 21, :],
                    in_=GT[:, 21 * g:21 * g + 21, :],
                ).then_inc(s_st, 16)
            sync.wait_ge(s_st, 64)

        @block.scalar
        def _(sc):
            sc.dma_start(
                out=hmB[1][:].rearrange("p (one rhi) c -> p one rhi c",
                                        one=1),
                in_=hm_blk[:, 1:2, :, :],
            ).then_inc(s_loadO, 16)
            sc.dma_start(out=statc[:], in_=statc_const[:]).then_inc(s_stt, 16)
            sc.dma_start(
                out=bias7w[:], in_=bias7w_const[:]).then_inc(s_stt, 16)
            sc.dma_start(
                out=hmB[3][:].rearrange("p (one rhi) c -> p one rhi c",
                                        one=1),
                in_=hm_blk[:, 3:4, :, :],
            ).then_inc(s_loadO, 16)
            for g in range(_NBLK):
                # DVE dtype-converting ops stall behind gpsimd descriptor
                # expansion, so ALL casts live on the Act engine:
                # idx_f = cast_f32(idxu); u = (idx+0.5)/7 - 0.5;
                # br = cast_i32(u) (round == floor after the -0.5 bias);
                # br_f = cast_f32(br)
                sc.wait_ge(s_topk, g + 1)
                sc.activation(
                    out=u_f[g][:], in_=idxu[g][:, 0:_NPATCH],
                    func=Act.Identity, scale=0.14285715, bias=bias07[:])
                sc.activation(
                    out=br_i[g][:], in_=u_f[g][:], func=Act.Identity)
                sc.activation(
                    out=br_f[g][:], in_=br_i[g][:], func=Act.Identity)
                sc.activation(
                    out=idx_f[g][:], in_=idxu[g][:, 0:_NPATCH],
                    func=Act.Identity)
                sc.drain().then_inc(s_brf, 1)
                # cast psD (PSUM, exact integers) + 7*(p%16) -> idx16 slice
                # (i16, SBUF); gpsimd cannot read PSUM, the Act engine can
                sc.wait_ge(s_bmm, g + 1)
                sl = slice(168 * g, 168 * g + 168)
                sc.activation(
                    out=idx16[:, sl], in_=psD[g][:], func=Act.Identity,
                    scale=1.0, bias=bias7w[:])
                sc.drain().then_inc(s_psd, 1)
            for g in (1, 3):
                sc.wait_ge(s_gc[g], 64)
                sc.dma_start(
                    out=out_pc[:, 21 * g:21 * g + 21, :],
                    in_=GT[:, 21 * g:21 * g + 21, :],
                ).then_inc(s_st, 16)


        @block.vector
        def _(vector):
            # constants (disjoint writes, no deps)
            vector.memset(G64[0:64, 0:1], 1.0)
            vector.memset(G64[0:64, 1:2], 0.0)
            vector.memset(G64[64:128, 0:1], 0.0)
            vector.memset(G64[64:128, 1:2], 1.0)
            vector.memset(ones128[:], 1.0)
            for b_ in range(2):
                vector.memset(sel1[b_][:, :], 0.0)
                vector.memset(sel1[b_][32 * b_:32 * b_ + 1, :], 1.0)
                vector.memset(sel441[b_][:, :], 0.0)
                vector.memset(sel441[b_][32 * b_:32 * b_ + 1, :], 441.0)
            vector.memset(warmidx[:], 0)
            # DVE CAST f32->i32 rounds to nearest: floor(x) == round(x - 0.5)
            # u = (idx + 0.5)/7 - 0.5 keeps >=0.07 margin from half-integers
            vector.memset(bias07[:], 0.071428575 - 0.5)
            vector.drain().then_inc(s_ones, 1)

            def R_stage(g):
                if g % 2 == 0:
                    vector.wait_ge(s_loadE, 16 * (g // 2 + 1))
                else:
                    vector.wait_ge(s_loadO, 16 * (g // 2 + 1))
                vector.reduce_sum(
                    out=red[g][:],
                    in_=hmB[g][:].rearrange("p rhi (bc u) -> p rhi bc u",
                                            u=64),
                    axis=X,
                )
                vector.drain().then_inc(s_red, 1)

            def T_stage(g):
                # two top-24 dances (one per batch) on [1,49] slices of
                # psV_g, rounds interleaved to hide drain latency
                vector.wait_ge(s_pmm, g + 1)
                vector.tensor_copy(
                    out=Vt[:].rearrange("p (br bc) -> p br bc", br=7),
                    in_=(psV[g][:]
                         .rearrange("p (br bc8) -> p br bc8",
                                    br=8)[:, 0:7, 0:7]),
                )
                vector.drain()
                cur = Vt
                for r3 in range(3):
                    vector.max(out=m2[:], in_=cur[:])
                    vector.drain()
                    vector.max_index(
                        out=idxu[g][:, 8 * r3:8 * r3 + 8], in_max=m2[:],
                        in_values=cur[:])
                    if r3 < 2:
                        nxt = vw[r3]
                        vector.match_replace(
                            out=nxt[:], in_to_replace=m2[:], in_values=cur[:],
                            imm_value=-1e30)
                        vector.drain()
                        cur = nxt
                vector.drain().then_inc(s_topk, 1)

            # reduce one block ahead so each P matmul (and its semaphore
            # hop) overlaps the next reduce instead of sitting between
            # R_g and T_g on the critical path
            R_stage(0)
            R_stage(1)
            T_stage(0)
            R_stage(2)
            T_stage(1)
            R_stage(3)
            T_stage(2)
            T_stage(3)

        @block.tensor
        def _(tensor):
            def P_stage(g):
                # interleaved rows: pooled group q = 2n + p//64 = 7b + br.
                # Four masked matmuls (per batch b x partition-half g2)
                # write psV2[g] [1,128] at 64b + 8*br + bc (pad unread).
                tensor.wait_ge(s_red, g + 1)
                if g == 0:
                    tensor.wait_ge(s_ones, 1)
                pieces = [
                    (0, 0, 0, 4, 0),   # b, g2, n0, cnt, two (br = 2n+g2-7b)
                    (0, 1, 0, 3, 1),
                    (1, 0, 4, 3, 1),
                    (1, 1, 3, 4, 0),
                ]
                for i, (b, g2, n0, cnt, two) in enumerate(pieces):
                    hb = (psV[g][32 * b:32 * b + 1, :]
                          .rearrange("p (n two bc8) -> p n two bc8",
                                     n=4, two=2, bc8=8))
                    tensor.matmul(
                        out=hb[:, 0:cnt, two:two + 1, 0:7],
                        lhsT=G64[:, g2:g2 + 1],
                        rhs=(red[g][:, n0:n0 + cnt, :]
                             .rearrange("p n (one bc) -> p n one bc",
                                        one=1)),
                        start=True, stop=True,
                    ).then_maybe_inc((s_pmm, 1) if i == 3 else None)

            def B_stage(g):
                # psD[g] = idx + 441*br (x4 along free) + static s-term
                tensor.wait_ge(s_brf, g + 1)
                if g == 0:
                    tensor.wait_ge(s_stt, 32)
                ridx = (idx_f[g][:]
                        .rearrange("p (m one) -> p m one", one=1)
                        .to_broadcast([33, _NPATCH, 4]))
                rbr = (br_f[g][:]
                       .rearrange("p (m one) -> p m one", one=1)
                       .to_broadcast([33, _NPATCH, 4]))
                for b in range(2):
                    sl = slice(84 * b, 84 * b + 84)
                    m_ = 2 * g + b
                    tensor.matmul(
                        out=psD[g][:, sl], lhsT=sel1[b][:], rhs=ridx,
                        start=True, stop=False)
                    tensor.matmul(
                        out=psD[g][:, sl], lhsT=sel441[b][:], rhs=rbr,
                        start=False, stop=False)
                    tensor.matmul(
                        out=psD[g][:, sl], lhsT=ones128[:],
                        rhs=statc[0:1, 84 * m_:84 * m_ + 84],
                        start=False, stop=True,
                    ).then_maybe_inc((s_bmm, 1) if b == 1 else None)

            P_stage(0)
            B_stage(0)
            P_stage(1)
            B_stage(1)
            P_stage(2)
            B_stage(2)
            P_stage(3)
            B_stage(3)

        @block.gpsimd
        def _(g):
            # preload the extended-instruction library early so the ucode
            # overlay DMA overlaps the heatmap phase
            from concourse import library_config
            g.load_library(library_config.mlp)
            # dummy gather absorbs any one-time ucode init cost
            g.wait_ge(s_ones, 1)
            g.dma_gather(
                out_ap=GTwarm[:],
                in_ap=img_rows,
                idxs_ap=warmidx[:],
                num_idxs=128,
                num_idxs_reg=128,
                elem_size=_PROW,
                queue_num=0,
            ).then_inc(s_warm, 16)
            g.wait_ge(s_warm, 16)
            # 4 calls per block on queues 0-3 so all four SWDGE core-pairs
            # expand descriptors concurrently
            for blk in range(_NBLK):
                g.wait_ge(s_psd, blk + 1)
                for c in range(4):
                    n = 128 * _CALL_COLS[c]
                    lo = 21 * blk + _CALL_OFF[c]
                    ilo = 168 * blk + 8 * _CALL_OFF[c]
                    g.dma_gather(
                        out_ap=GT[:, lo:lo + _CALL_COLS[c], :],
                        in_ap=img_rows,
                        idxs_ap=idx16[:, ilo:ilo + 8 * _CALL_COLS[c]],
                        num_idxs=n,
                        num_idxs_reg=n,
                        elem_size=_PROW,
                        queue_num=(c + blk) % 4,
                    ).then_inc(s_gc[blk], 16)

    nc.finalize()
    _nc_cache = nc
    return nc


def kernel(heatmap, image):
    from concourse.bass_utils import run_bass_kernel_spmd

    heatmap = np.ascontiguousarray(np.asarray(heatmap), dtype=np.float32)
    image = np.ascontiguousarray(np.asarray(image), dtype=np.float32)
    assert heatmap.shape == (_B, 448, 448, 1)
    assert image.shape == (_B, 448, 448, 3)

    nc = build_program()
    in_maps = [
        {
            "heatmap": heatmap[c * _B_LOC:(c + 1) * _B_LOC],
            "image": image[c * _B_LOC:(c + 1) * _B_LOC],
        }
        for c in range(_N_CORES)
    ]
    res = run_bass_kernel_spmd(nc, in_maps, list(range(_N_CORES)))
    outs = [res.results[c]["out"] for c in range(_N_CORES)]
    return np.concatenate(outs, axis=0)
